# revision 2
# baseline (speedup 1.0000x reference)
"""3-layer LSTM (B=256,T=512,I=256,H=512) + linear head on 8 NeuronCores.

Strategy: data-parallel over batch (32/core). Per layer, the input-side
matmul G = Wih @ x_t (+ biases) for a *chunk* of future time steps is
computed at full PE efficiency (N=512 streams) and interleaved with the
sequential h-recurrence of the current chunk; G never leaves SBUF.
Gate layout: gates.T packed [128 part, 512 cols] = 16 slots of 32 batch
cols in slot order g|i|f|o, all in ONE PSUM bank per step, preloaded
with G via one DVE copy, accumulated by 64 weight-stationary bf16
matmuls (K=128, M=128, N=32). ACT evacuation is split per gate block so
tanh(g)/sigmoid(i,f,o) start as soon as that gate's 4 slots finish,
overlapping the c/h elementwise chain with the tail of the matmul
block. The chunk loop is unrolled 4x inside the hardware loop so all
SBUF access patterns are static (no per-matmul register-AP setup on the
PE queue). c stays fp32-resident; h is written bf16 per feature block
(block 0 first, unblocking the next step's first matmul) into the
layout the next matmul and the next layer's batched input matmul
consume.
"""

import numpy as np
import ml_dtypes
from contextlib import ExitStack

import concourse.bass as bass
import concourse.bacc as bacc
import concourse.tile as tile
from concourse import mybir
from concourse.bass_utils import run_bass_kernel_spmd

BF16 = mybir.dt.bfloat16
F32 = mybir.dt.float32
AF = mybir.ActivationFunctionType

B, T, I, H, O = 256, 512, 256, 512, 3
NCORES = 8
BL = B // NCORES          # 32 batch rows per core
SC = 16                   # time steps per chunk
CW = SC * BL              # 512 cols per chunk
NCH = T // SC             # 32 chunks
UNROLL = 4                # chunks per hardware-loop iteration
TOT = T * BL              # 16384 cols total
SLACK = 2 * CW            # prefetch overrun slack (cols)

# gate blocks in psum-slot order: g | i | f | o  (slot = blk*4 + j)
# block -> base row in the canonical (i,f,g,o) 2048 gate layout
GATE_BASE = [1024, 0, 512, 1536]   # g, i, f, o
KCS = [2, 4, 4]                    # K chunks per layer (256, 512, 512)


def _slot_row(slot):
    return GATE_BASE[slot // 4] + 128 * (slot % 4)


def _build():
    nc = bacc.Bacc("TRN2", target_bir_lowering=False, debug=False,
                   num_devices=NCORES)

    xt = nc.dram_tensor("x_t", (128, 2, TOT + SLACK), BF16, kind="ExternalInput")
    wih = [nc.dram_tensor(f"wih{l}", (128, KCS[l] * 2048), BF16,
                          kind="ExternalInput") for l in range(3)]
    whh = [nc.dram_tensor(f"whh{l}", (128, 4 * 2048), BF16,
                          kind="ExternalInput") for l in range(3)]
    bias_d = nc.dram_tensor("bias", (128, 48), F32, kind="ExternalInput")
    fcw_d = nc.dram_tensor("fcw", (128, 12), BF16, kind="ExternalInput")
    fcb_d = nc.dram_tensor("fcb", (3, 1), F32, kind="ExternalInput")
    out_d = nc.dram_tensor("out", (3, BL), F32, kind="ExternalOutput")

    with tile.TileContext(nc) as tc, ExitStack() as ctx:
        dram = ctx.enter_context(tc.tile_pool(name="dram", bufs=1, space="DRAM"))
        hdr = dram.tile([128, 4, TOT + SLACK], BF16)   # inter-layer H seq

        const = ctx.enter_context(tc.tile_pool(name="const", bufs=1))
        wih_sb = [const.tile([128, KCS[l] * 2048], BF16, tag=f"wih{l}",
                             name=f"wih_sb{l}") for l in range(3)]
        whh_sb = [const.tile([128, 4 * 2048], BF16, tag=f"whh{l}",
                             name=f"whh_sb{l}") for l in range(3)]
        bias_sb = const.tile([128, 48], F32, tag="bias")
        fcw_sb = const.tile([128, 12], BF16, tag="fcw")
        fcb_sb = const.tile([3, 1], F32, tag="fcb")
        for l in range(3):
            nc.sync.dma_start(wih_sb[l][:], wih[l].ap())
            nc.sync.dma_start(whh_sb[l][:], whh[l].ap())
        nc.sync.dma_start(bias_sb[:], bias_d.ap())
        nc.sync.dma_start(fcw_sb[:], fcw_d.ap())
        nc.sync.dma_start(fcb_sb[:], fcb_d.ap())

        big = ctx.enter_context(tc.tile_pool(name="big", bufs=1))
        g_buf = big.tile([128, 2 * 16 * CW], BF16, tag="gbuf")     # 4MB
        in_buf = big.tile([128, 4 * 4 * CW], BF16, tag="inbuf")    # 2MB
        h_stage = big.tile([128, 2 * 4 * CW], BF16, tag="hstage")  # 1MB
        c_t = big.tile([128, 128], F32, tag="cstate")

        g3 = g_buf[:].rearrange("p (s c) -> p s c", c=CW)    # [128, 32, CW]
        i3 = in_buf[:].rearrange("p (b c) -> p b c", c=CW)   # [128, 16, CW]
        h3 = h_stage[:].rearrange("p (x c) -> p x c", c=CW)  # [128, 8, CW]

        ew = ctx.enter_context(tc.tile_pool(name="ew", bufs=2))
        ps_rec = ctx.enter_context(tc.tile_pool(name="psr", bufs=2, space="PSUM"))
        ps_pa = ctx.enter_context(tc.tile_pool(name="psa", bufs=2, space="PSUM"))
        ps_fc = ctx.enter_context(tc.tile_pool(name="psf", bufs=1, space="PSUM"))

        def phase_a_slot(l, slot, in_base, g_base, in_ap):
            """G[slot] for one chunk: Kc matmuls (N=CW) + bias ACT."""
            kc = KCS[l]
            ps = ps_pa.tile([128, CW], F32, tag="pa")
            for k in range(kc):
                nc.tensor.matmul(
                    ps[:],
                    lhsT=wih_sb[l][:, k * 2048 + _slot_row(slot):
                                   k * 2048 + _slot_row(slot) + 128],
                    rhs=in_ap(in_base + k),
                    start=(k == 0), stop=(k == kc - 1),
                )
            nc.scalar.activation(
                g3[:, bass.ds(g_base + slot, 1), :].rearrange("p a c -> p (a c)"),
                ps[:], AF.Identity, bias=bias_sb[:, l * 16 + slot: l * 16 + slot + 1])

        def rec_step(l, s, g_base, h_rd, h_wr, pa_emit):
            """One recurrence time step; h_rd/h_wr are h3 block bases.

            Slot order is g(0-3) i(4-7) f(8-11) o(12-15); after each gate's
            4 slots finish, its activation is issued so the c/h chain
            overlaps the remaining matmuls.
            """
            ps = ps_rec.tile([128, 512], F32, tag="rec")
            nc.vector.tensor_copy(
                ps[:].rearrange("p (a b) -> p a b", b=BL),
                g3[:, bass.ds(g_base, 16), s * BL: (s + 1) * BL])
            # h[t-1]: last slot of the other-parity buffer for s=0,
            # else slot s-1 of the current chunk's buffer
            hp_base = h_rd if s == 0 else h_wr
            hp_col = ((SC - 1) if s == 0 else (s - 1)) * BL
            gt = ew.tile([128, 512], F32, tag="gates")
            for blk in range(4):
                for j in range(4):
                    slot = blk * 4 + j
                    for k in range(4):
                        nc.tensor.matmul(
                            ps[:, slot * BL:(slot + 1) * BL],
                            lhsT=whh_sb[l][:, k * 2048 + _slot_row(slot):
                                           k * 2048 + _slot_row(slot) + 128],
                            rhs=h3[:, bass.ds(hp_base + k, 1),
                                   hp_col:hp_col + BL].rearrange("p a c -> p (a c)"),
                            start=False, stop=(k == 3), skip_group_check=True,
                        )
                # gate block done -> evacuate/activate it now
                nc.scalar.activation(
                    gt[:, blk * 128:(blk + 1) * 128],
                    ps[:, blk * 128:(blk + 1) * 128],
                    AF.Tanh if blk == 0 else AF.Sigmoid)
            t1 = ew.tile([128, 128], F32, tag="t1")
            t2 = ew.tile([128, 128], F32, tag="t2")
            nc.vector.tensor_mul(t1[:], gt[:, 128:256], gt[:, 0:128])     # i*g
            nc.vector.tensor_mul(t2[:], gt[:, 256:384], c_t[:])           # f*c
            nc.vector.tensor_add(c_t[:], t1[:], t2[:])
            th = ew.tile([128, 128], F32, tag="th")
            nc.scalar.activation(th[:], c_t[:], AF.Tanh)
            # h per feature block, block 0 first: unblocks the next step's
            # first matmul (slot 0, k=0) as early as possible
            for b in range(4):
                nc.vector.tensor_mul(
                    h3[:, bass.ds(h_wr + b, 1), s * BL:(s + 1) * BL]
                    .rearrange("p a c -> p (a c)"),
                    gt[:, 384 + b * BL:384 + (b + 1) * BL],
                    th[:, b * BL:(b + 1) * BL])
            if pa_emit is not None:
                pa_emit(s)

        for l in range(3):
            in_dram = xt.ap() if l == 0 else hdr[:]
            kc = KCS[l]

            # prologue: In chunks 0,1 -> bufs 0,1 ; G chunk 0 -> parity 0
            nc.sync.dma_start(i3[:, 0:kc, :], in_dram[:, :, 0:CW])
            nc.sync.dma_start(i3[:, kc:2 * kc, :], in_dram[:, :, CW:2 * CW])
            for slot in range(16):
                phase_a_slot(l, slot, 0, 0,
                             lambda idx: i3[:, bass.ds(idx, 1), :]
                             .rearrange("p a c -> p (a c)"))
            nc.vector.memset(c_t[:], 0.0)
            nc.vector.memset(h3[:, bass.ds(4, 4), (SC - 1) * BL: SC * BL], 0.0)

            def body(ov, u, l=l, kc=kc, in_dram=in_dram):
                # chunk index civ = ov*UNROLL + u; all mod-2/mod-4 parities
                # depend only on u -> compile-time static APs
                p2 = u & 1
                q2 = (u + 1) & 1
                ld_buf = ((u + 2) & 3) * kc
                use_buf = ((u + 1) & 3) * kc
                nc.sync.dma_start(
                    i3[:, bass.ds(ld_buf, kc), :],
                    in_dram[:, :, bass.ds((ov * UNROLL + u + 2) * CW, CW)])

                def pa_emit(s, l=l, use_buf=use_buf, q2=q2):
                    phase_a_slot(l, s, use_buf, q2 * 16,
                                 lambda idx: i3[:, bass.ds(idx, 1), :]
                                 .rearrange("p a c -> p (a c)"))

                for s in range(SC):
                    rec_step(l, s, p2 * 16, q2 * 4, p2 * 4, pa_emit)
                if l < 2:
                    nc.sync.dma_start(
                        hdr[:, :, bass.ds((ov * UNROLL + u) * CW, CW)],
                        h3[:, bass.ds(p2 * 4, 4), :])

            with tc.For_i(0, NCH // UNROLL, 1) as ov:
                for u in range(UNROLL):
                    body(ov, u)

        # final linear head: out.T [3, BL] = fcW @ h_last (+ fcB)
        hb = ((NCH - 1) & 1) * 4
        ps = ps_fc.tile([3, BL], F32, tag="fc")
        for k in range(4):
            nc.tensor.matmul(
                ps[:], lhsT=fcw_sb[:, k * 3:(k + 1) * 3],
                rhs=h3[:, bass.ds(hb + k, 1), (SC - 1) * BL: SC * BL]
                .rearrange("p a c -> p (a c)"),
                start=(k == 0), stop=(k == 3))
        ob = ew.tile([3, BL], F32, tag="out")
        nc.scalar.activation(ob[:], ps[:], AF.Identity, bias=fcb_sb[:])
        nc.sync.dma_start(out_d.ap(), ob[:])

    nc.compile()
    return nc


def _prep(inputs):
    """Host-side layout prep. Returns per-core in_maps."""
    bf = ml_dtypes.bfloat16
    x = np.asarray(inputs["x"], np.float32)
    wihs = [np.asarray(inputs[f"Wih{l}"], np.float32) for l in range(3)]
    whhs = [np.asarray(inputs[f"Whh{l}"], np.float32) for l in range(3)]

    def wt_pack(w, kcs):  # [2048, K] -> [128, kcs*2048]
        return np.ascontiguousarray(
            w.T.reshape(kcs, 128, 2048).transpose(1, 0, 2)
            .reshape(128, kcs * 2048)).astype(bf)

    shared = {}
    for l in range(3):
        shared[f"wih{l}"] = wt_pack(wihs[l], KCS[l])
        shared[f"whh{l}"] = wt_pack(whhs[l], 4)
    bias = np.zeros((128, 48), np.float32)
    for l in range(3):
        bl_ = (np.asarray(inputs[f"bih{l}"], np.float32)
               + np.asarray(inputs[f"bhh{l}"], np.float32))
        for slot in range(16):
            r = _slot_row(slot)
            bias[:, l * 16 + slot] = bl_[r:r + 128]
    shared["bias"] = bias
    shared["fcw"] = np.ascontiguousarray(
        np.asarray(inputs["fcW"], np.float32).T.reshape(4, 128, 3)
        .transpose(1, 0, 2).reshape(128, 12)).astype(bf)
    shared["fcb"] = np.asarray(inputs["fcB"], np.float32).reshape(3, 1)

    in_maps = []
    for c in range(NCORES):
        xc = x[c * BL:(c + 1) * BL]                       # [32, 512, 256]
        xp = xc.transpose(2, 1, 0).reshape(2, 128, TOT)   # [2,128,16384]
        xp = np.ascontiguousarray(xp.transpose(1, 0, 2))  # [128,2,16384]
        xp = np.concatenate(
            [xp, np.zeros((128, 2, SLACK), np.float32)], axis=2).astype(bf)
        in_maps.append({"x_t": xp, **shared})
    return in_maps


_NC_CACHE = None


def kernel(**inputs):
    global _NC_CACHE
    if _NC_CACHE is None:
        _NC_CACHE = _build()
    nc = _NC_CACHE
    in_maps = _prep(inputs)
    res = run_bass_kernel_spmd(nc, in_maps, core_ids=list(range(NCORES)))
    out = np.empty((B, O), np.float32)
    for c in range(NCORES):
        out[c * BL:(c + 1) * BL] = res.results[c]["out"].T
    return out


# revision 14
# speedup vs baseline: 1.1364x; 1.1364x over previous
"""3-layer LSTM (B=256,T=512,I=256,H=512) + linear head on 8 NeuronCores.

Strategy: data-parallel over batch (32/core). Per layer, the input-side
matmul G = Wih @ x_t (+ biases) for a *chunk* of future time steps is
computed at full PE efficiency (N=512 streams) and interleaved with the
sequential h-recurrence of the current chunk; G never leaves SBUF.
Gate layout: gates.T packed as 4 PSUM tiles (one per gate g|i|f|o) of
[128 part, 128 cols = 4 feature slots x 32 batch] per step. Separate
tiles keep the per-gate ACT evacuations from serializing against the
next gate's matmuls. G is preloaded into PSUM by identity matmuls
(I.T @ G, start=True) on the PE itself -- no DVE copy on the critical
path, and the 16 preload matmuls pad the PE pipe while the previous
step's h chain drains. Each step then runs 64 weight-stationary bf16
accumulate matmuls (K=128, M=128, N=32). ACT evacuation is split per
gate block (tanh g, sigmoid i, f) and per slot for o, with tanh(c)
issued between f and o, so the c/h elementwise chain overlaps the tail
of the matmul block and h feature block 0 lands as early as possible.
The chunk loop is unrolled 4x inside the hardware loop so all SBUF
access patterns are static (no per-matmul register-AP setup on the PE
queue). c stays fp32-resident; h is written bf16 per feature block
into the layout the next matmul and the next layer's batched input
matmul consume.
"""

import numpy as np
import ml_dtypes
from contextlib import ExitStack

import concourse.bass as bass
import concourse.bacc as bacc
import concourse.tile as tile
from concourse import mybir
from concourse.bass_utils import run_bass_kernel_spmd

BF16 = mybir.dt.bfloat16
F32 = mybir.dt.float32
AF = mybir.ActivationFunctionType

B, T, I, H, O = 256, 512, 256, 512, 3
NCORES = 8
BL = B // NCORES          # 32 batch rows per core
SC = 16                   # time steps per chunk
CW = SC * BL              # 512 cols per chunk
NCH = T // SC             # 32 chunks
UNROLL = 4                # chunks per hardware-loop iteration
TOT = T * BL              # 16384 cols total
SLACK = 2 * CW            # prefetch overrun slack (cols)

# gate blocks in psum-slot order: g | i | f | o  (slot = blk*4 + j)
# block -> base row in the canonical (i,f,g,o) 2048 gate layout
GATE_BASE = [1024, 0, 512, 1536]   # g, i, f, o
KCS = [2, 4, 4]                    # K chunks per layer (256, 512, 512)


def _slot_row(slot):
    return GATE_BASE[slot // 4] + 128 * (slot % 4)


def _build():
    nc = bacc.Bacc("TRN2", target_bir_lowering=False, debug=False,
                   num_devices=NCORES)

    xt = nc.dram_tensor("x_t", (128, 2, TOT + SLACK), BF16, kind="ExternalInput")
    wih = [nc.dram_tensor(f"wih{l}", (128, KCS[l] * 2048), BF16,
                          kind="ExternalInput") for l in range(3)]
    whh = [nc.dram_tensor(f"whh{l}", (128, 4 * 2048), BF16,
                          kind="ExternalInput") for l in range(3)]
    bias_d = nc.dram_tensor("bias", (128, 48), F32, kind="ExternalInput")
    ident_d = nc.dram_tensor("ident", (128, 128), BF16, kind="ExternalInput")
    fcw_d = nc.dram_tensor("fcw", (128, 12), BF16, kind="ExternalInput")
    fcb_d = nc.dram_tensor("fcb", (3, 1), F32, kind="ExternalInput")
    out_d = nc.dram_tensor("out", (3, BL), F32, kind="ExternalOutput")

    with tile.TileContext(nc) as tc, ExitStack() as ctx:
        dram = ctx.enter_context(tc.tile_pool(name="dram", bufs=1, space="DRAM"))
        hdr = dram.tile([128, 4, TOT + SLACK], BF16)   # inter-layer H seq

        const = ctx.enter_context(tc.tile_pool(name="const", bufs=1))
        wih_sb = [const.tile([128, KCS[l] * 2048], BF16, tag=f"wih{l}",
                             name=f"wih_sb{l}") for l in range(3)]
        whh_sb = [const.tile([128, 4 * 2048], BF16, tag=f"whh{l}",
                             name=f"whh_sb{l}") for l in range(3)]
        bias_sb = const.tile([128, 48], F32, tag="bias")
        ident_sb = const.tile([128, 128], BF16, tag="ident")
        fcw_sb = const.tile([128, 12], BF16, tag="fcw")
        fcb_sb = const.tile([3, 1], F32, tag="fcb")
        for l in range(3):
            nc.sync.dma_start(wih_sb[l][:], wih[l].ap())
            nc.sync.dma_start(whh_sb[l][:], whh[l].ap())
        nc.sync.dma_start(bias_sb[:], bias_d.ap())
        nc.sync.dma_start(ident_sb[:], ident_d.ap())
        nc.sync.dma_start(fcw_sb[:], fcw_d.ap())
        nc.sync.dma_start(fcb_sb[:], fcb_d.ap())

        big = ctx.enter_context(tc.tile_pool(name="big", bufs=1))
        g_buf = big.tile([128, 2 * 16 * CW], BF16, tag="gbuf")     # 4MB
        in_buf = big.tile([128, 4 * 4 * CW], BF16, tag="inbuf")    # 2MB
        h_stage = big.tile([128, 2 * 4 * CW], BF16, tag="hstage")  # 1MB
        c_t = big.tile([128, 128], F32, tag="cstate")

        g3 = g_buf[:].rearrange("p (s c) -> p s c", c=CW)    # [128, 32, CW]
        # slot = grp*8 + gi*4 + j: gi=0 -> gates g,f (psum tile0),
        # gi=1 -> gates i,o (tile1)
        g6 = g_buf[:].rearrange("p (par grp gi j c) -> p par grp gi j c",
                                par=2, grp=2, gi=2, j=4)
        i3 = in_buf[:].rearrange("p (b c) -> p b c", c=CW)   # [128, 16, CW]
        h3 = h_stage[:].rearrange("p (x c) -> p x c", c=CW)  # [128, 8, CW]

        ew = ctx.enter_context(tc.tile_pool(name="ew", bufs=2))
        ps_rec = ctx.enter_context(tc.tile_pool(name="psr", bufs=2, space="PSUM"))
        ps_pa = ctx.enter_context(tc.tile_pool(name="psa", bufs=2, space="PSUM"))
        ps_fc = ctx.enter_context(tc.tile_pool(name="psf", bufs=1, space="PSUM"))

        def phase_a_slot(l, slot, in_base, g_base, in_ap):
            """G[slot] for one chunk: Kc matmuls (N=CW) + bias ACT."""
            kc = KCS[l]
            ps = ps_pa.tile([128, CW], F32, tag="pa")
            for k in range(kc):
                nc.tensor.matmul(
                    ps[:],
                    lhsT=wih_sb[l][:, k * 2048 + _slot_row(slot):
                                   k * 2048 + _slot_row(slot) + 128],
                    rhs=in_ap(in_base + k),
                    start=(k == 0), stop=(k == kc - 1),
                )
            nc.scalar.activation(
                g3[:, bass.ds(g_base + slot, 1), :].rearrange("p a c -> p (a c)"),
                ps[:], AF.Identity, bias=bias_sb[:, l * 16 + slot: l * 16 + slot + 1])

        def rec_step(l, s, g_base, h_rd, h_wr, pa_emit):
            """One recurrence time step; h_rd/h_wr are h3 block bases.

            Slot order is g(0-3) i(4-7) f(8-11) o(12-15); after each gate's
            4 slots finish, its activation is issued so the c/h chain
            overlaps the remaining matmuls.
            """
            # Two PSUM tiles per step: tile0 = g|f, tile1 = i|o. The pairing
            # interleaves so each gate's ACT evacuation overlaps the other
            # tile's matmul block instead of stalling its own tile's writers.
            ps0 = ps_rec.tile([128, 256], F32, tag="rec0", name="ps0")
            ps1 = ps_rec.tile([128, 256], F32, tag="rec1", name="ps1")
            # blk (g,i,f,o) -> (tile, col base)
            pmap = [(ps0, 0), (ps1, 0), (ps0, 128), (ps1, 128)]
            # PSUM preload G via identity matmuls: no h dependency, so these
            # matmuls keep the PE busy while the previous step's h chain
            # drains. Exactly ONE start=True matmul per tile: start marks the
            # whole 2KB PSUM bank pending-zero, so a second start in the same
            # bank would wipe the first preload's values.
            par = g_base // 16
            for t, pt in enumerate((ps0, ps1)):
                nc.tensor.matmul(
                    pt[:].rearrange("p (g a c) -> p g a c", g=2, c=BL),
                    lhsT=ident_sb[:],
                    rhs=g6[:, bass.ds(par, 1), :, bass.ds(t, 1), :,
                           s * BL:(s + 1) * BL],
                    start=True, stop=False, skip_group_check=True,
                )
            # h[t-1]: last slot of the other-parity buffer for s=0,
            # else slot s-1 of the current chunk's buffer
            hp_base = h_rd if s == 0 else h_wr
            hp_col = ((SC - 1) if s == 0 else (s - 1)) * BL
            gt = ew.tile([128, 512], F32, tag="gates")
            t1 = ew.tile([128, 128], F32, tag="t1")
            t2 = ew.tile([128, 128], F32, tag="t2")
            th = ew.tile([128, 128], F32, tag="th")

            def acc_slot(slot):
                blk, j = slot // 4, slot % 4
                pt, pc = pmap[blk]
                for k in range(4):
                    nc.tensor.matmul(
                        pt[:, pc + j * BL:pc + (j + 1) * BL],
                        lhsT=whh_sb[l][:, k * 2048 + _slot_row(slot):
                                       k * 2048 + _slot_row(slot) + 128],
                        rhs=h3[:, bass.ds(hp_base + k, 1),
                               hp_col:hp_col + BL].rearrange("p a c -> p (a c)"),
                        start=False, stop=(k == 3), skip_group_check=True,
                    )

            for slot in range(4):                                         # g
                acc_slot(slot)
            nc.scalar.activation(gt[:, 0:128], ps0[:, 0:128], AF.Tanh)
            for slot in range(4, 8):                                      # i
                acc_slot(slot)
            nc.scalar.activation(gt[:, 128:256], ps1[:, 0:128], AF.Sigmoid)
            nc.vector.tensor_mul(t1[:], gt[:, 128:256], gt[:, 0:128])     # i*g
            for slot in range(8, 12):                                     # f
                acc_slot(slot)
            nc.scalar.activation(gt[:, 256:384], ps0[:, 128:256], AF.Sigmoid)
            nc.vector.tensor_mul(t2[:], gt[:, 256:384], c_t[:])           # f*c
            nc.vector.tensor_add(c_t[:], t1[:], t2[:])
            nc.scalar.activation(th[:], c_t[:], AF.Tanh)
            # o: sigmoid per slot + h per feature block right after that
            # slot's matmuls; block 0 first, unblocking the next step's
            # first matmul early
            for b in range(4):
                acc_slot(12 + b)
                nc.scalar.activation(gt[:, 384 + b * BL:384 + (b + 1) * BL],
                                     ps1[:, 128 + b * BL:128 + (b + 1) * BL],
                                     AF.Sigmoid)
                nc.vector.tensor_mul(
                    h3[:, bass.ds(h_wr + b, 1), s * BL:(s + 1) * BL]
                    .rearrange("p a c -> p (a c)"),
                    gt[:, 384 + b * BL:384 + (b + 1) * BL],
                    th[:, b * BL:(b + 1) * BL])
            if pa_emit is not None:
                pa_emit(s)

        for l in range(3):
            in_dram = xt.ap() if l == 0 else hdr[:]
            kc = KCS[l]

            # prologue: In chunks 0,1 -> bufs 0,1 ; G chunk 0 -> parity 0
            nc.sync.dma_start(i3[:, 0:kc, :], in_dram[:, :, 0:CW])
            nc.sync.dma_start(i3[:, kc:2 * kc, :], in_dram[:, :, CW:2 * CW])
            for slot in range(16):
                phase_a_slot(l, slot, 0, 0,
                             lambda idx: i3[:, bass.ds(idx, 1), :]
                             .rearrange("p a c -> p (a c)"))
            nc.vector.memset(c_t[:], 0.0)
            nc.vector.memset(h3[:, bass.ds(4, 4), (SC - 1) * BL: SC * BL], 0.0)

            def body(ov, u, l=l, kc=kc, in_dram=in_dram):
                # chunk index civ = ov*UNROLL + u; all mod-2/mod-4 parities
                # depend only on u -> compile-time static APs
                p2 = u & 1
                q2 = (u + 1) & 1
                ld_buf = ((u + 2) & 3) * kc
                use_buf = ((u + 1) & 3) * kc
                nc.sync.dma_start(
                    i3[:, bass.ds(ld_buf, kc), :],
                    in_dram[:, :, bass.ds((ov * UNROLL + u + 2) * CW, CW)])

                def pa_emit(s, l=l, use_buf=use_buf, q2=q2):
                    phase_a_slot(l, s, use_buf, q2 * 16,
                                 lambda idx: i3[:, bass.ds(idx, 1), :]
                                 .rearrange("p a c -> p (a c)"))

                for s in range(SC):
                    rec_step(l, s, p2 * 16, q2 * 4, p2 * 4, pa_emit)
                if l < 2:
                    nc.sync.dma_start(
                        hdr[:, :, bass.ds((ov * UNROLL + u) * CW, CW)],
                        h3[:, bass.ds(p2 * 4, 4), :])

            with tc.For_i(0, NCH // UNROLL, 1) as ov:
                for u in range(UNROLL):
                    body(ov, u)

        # final linear head: out.T [3, BL] = fcW @ h_last (+ fcB)
        hb = ((NCH - 1) & 1) * 4
        ps = ps_fc.tile([3, BL], F32, tag="fc")
        for k in range(4):
            nc.tensor.matmul(
                ps[:], lhsT=fcw_sb[:, k * 3:(k + 1) * 3],
                rhs=h3[:, bass.ds(hb + k, 1), (SC - 1) * BL: SC * BL]
                .rearrange("p a c -> p (a c)"),
                start=(k == 0), stop=(k == 3))
        ob = ew.tile([3, BL], F32, tag="out")
        nc.scalar.activation(ob[:], ps[:], AF.Identity, bias=fcb_sb[:])
        nc.sync.dma_start(out_d.ap(), ob[:])

    nc.compile()
    return nc


def _prep(inputs):
    """Host-side layout prep. Returns per-core in_maps."""
    bf = ml_dtypes.bfloat16
    x = np.asarray(inputs["x"], np.float32)
    wihs = [np.asarray(inputs[f"Wih{l}"], np.float32) for l in range(3)]
    whhs = [np.asarray(inputs[f"Whh{l}"], np.float32) for l in range(3)]

    def wt_pack(w, kcs):  # [2048, K] -> [128, kcs*2048]
        return np.ascontiguousarray(
            w.T.reshape(kcs, 128, 2048).transpose(1, 0, 2)
            .reshape(128, kcs * 2048)).astype(bf)

    shared = {}
    for l in range(3):
        shared[f"wih{l}"] = wt_pack(wihs[l], KCS[l])
        shared[f"whh{l}"] = wt_pack(whhs[l], 4)
    shared["ident"] = np.eye(128, dtype=bf)
    bias = np.zeros((128, 48), np.float32)
    for l in range(3):
        bl_ = (np.asarray(inputs[f"bih{l}"], np.float32)
               + np.asarray(inputs[f"bhh{l}"], np.float32))
        for slot in range(16):
            r = _slot_row(slot)
            bias[:, l * 16 + slot] = bl_[r:r + 128]
    shared["bias"] = bias
    shared["fcw"] = np.ascontiguousarray(
        np.asarray(inputs["fcW"], np.float32).T.reshape(4, 128, 3)
        .transpose(1, 0, 2).reshape(128, 12)).astype(bf)
    shared["fcb"] = np.asarray(inputs["fcB"], np.float32).reshape(3, 1)

    in_maps = []
    for c in range(NCORES):
        xc = x[c * BL:(c + 1) * BL]                       # [32, 512, 256]
        xp = xc.transpose(2, 1, 0).reshape(2, 128, TOT)   # [2,128,16384]
        xp = np.ascontiguousarray(xp.transpose(1, 0, 2))  # [128,2,16384]
        xp = np.concatenate(
            [xp, np.zeros((128, 2, SLACK), np.float32)], axis=2).astype(bf)
        in_maps.append({"x_t": xp, **shared})
    return in_maps


_NC_CACHE = None


def kernel(**inputs):
    global _NC_CACHE
    if _NC_CACHE is None:
        _NC_CACHE = _build()
    nc = _NC_CACHE
    in_maps = _prep(inputs)
    res = run_bass_kernel_spmd(nc, in_maps, core_ids=list(range(NCORES)))
    out = np.empty((B, O), np.float32)
    for c in range(NCORES):
        out[c * BL:(c + 1) * BL] = res.results[c]["out"].T
    return out


# revision 16
# speedup vs baseline: 1.1368x; 1.0003x over previous
"""3-layer LSTM (B=256,T=512,I=256,H=512) + linear head on 8 NeuronCores.

Strategy: data-parallel over batch (32/core). Per layer, the input-side
matmul G = Wih @ x_t (+ biases) for a *chunk* of future time steps is
computed at full PE efficiency (N=512 streams) and interleaved with the
sequential h-recurrence of the current chunk; G never leaves SBUF.
Gate layout: gates.T packed as 4 PSUM tiles (one per gate g|i|f|o) of
[128 part, 128 cols = 4 feature slots x 32 batch] per step. Separate
tiles keep the per-gate ACT evacuations from serializing against the
next gate's matmuls. G is preloaded into PSUM by identity matmuls
(I.T @ G, start=True) on the PE itself -- no DVE copy on the critical
path, and the 16 preload matmuls pad the PE pipe while the previous
step's h chain drains. Each step then runs 64 weight-stationary bf16
accumulate matmuls (K=128, M=128, N=32). ACT evacuation is split per
gate block (tanh g, sigmoid i, f) and per slot for o, with tanh(c)
issued between f and o, so the c/h elementwise chain overlaps the tail
of the matmul block and h feature block 0 lands as early as possible.
The chunk loop is unrolled 4x inside the hardware loop so all SBUF
access patterns are static (no per-matmul register-AP setup on the PE
queue). c stays fp32-resident; h is written bf16 per feature block
into the layout the next matmul and the next layer's batched input
matmul consume.
"""

import numpy as np
import ml_dtypes
from contextlib import ExitStack

import concourse.bass as bass
import concourse.bacc as bacc
import concourse.tile as tile
from concourse import mybir
from concourse.bass_utils import run_bass_kernel_spmd

BF16 = mybir.dt.bfloat16
F32 = mybir.dt.float32
AF = mybir.ActivationFunctionType

B, T, I, H, O = 256, 512, 256, 512, 3
NCORES = 8
BL = B // NCORES          # 32 batch rows per core
SC = 16                   # time steps per chunk
CW = SC * BL              # 512 cols per chunk
NCH = T // SC             # 32 chunks
UNROLL = 4                # chunks per hardware-loop iteration
TOT = T * BL              # 16384 cols total
SLACK = 2 * CW            # prefetch overrun slack (cols)

# gate blocks in psum-slot order: g | i | f | o  (slot = blk*4 + j)
# block -> base row in the canonical (i,f,g,o) 2048 gate layout
GATE_BASE = [1024, 0, 512, 1536]   # g, i, f, o
KCS = [2, 4, 4]                    # K chunks per layer (256, 512, 512)


def _slot_row(slot):
    return GATE_BASE[slot // 4] + 128 * (slot % 4)


def _build():
    nc = bacc.Bacc("TRN2", target_bir_lowering=False, debug=False,
                   num_devices=NCORES)

    xt = nc.dram_tensor("x_t", (128, 2, TOT + SLACK), BF16, kind="ExternalInput")
    wih = [nc.dram_tensor(f"wih{l}", (128, KCS[l] * 2048), BF16,
                          kind="ExternalInput") for l in range(3)]
    whh = [nc.dram_tensor(f"whh{l}", (128, 4 * 2048), BF16,
                          kind="ExternalInput") for l in range(3)]
    bias_d = nc.dram_tensor("bias", (128, 48), F32, kind="ExternalInput")
    ident_d = nc.dram_tensor("ident", (128, 128), BF16, kind="ExternalInput")
    fcw_d = nc.dram_tensor("fcw", (128, 12), BF16, kind="ExternalInput")
    fcb_d = nc.dram_tensor("fcb", (3, 1), F32, kind="ExternalInput")
    out_d = nc.dram_tensor("out", (3, BL), F32, kind="ExternalOutput")

    with tile.TileContext(nc) as tc, ExitStack() as ctx:
        dram = ctx.enter_context(tc.tile_pool(name="dram", bufs=1, space="DRAM"))
        hdr = dram.tile([128, 4, TOT + SLACK], BF16)   # inter-layer H seq

        const = ctx.enter_context(tc.tile_pool(name="const", bufs=1))
        wih_sb = [const.tile([128, KCS[l] * 2048], BF16, tag=f"wih{l}",
                             name=f"wih_sb{l}") for l in range(3)]
        whh_sb = [const.tile([128, 4 * 2048], BF16, tag=f"whh{l}",
                             name=f"whh_sb{l}") for l in range(3)]
        bias_sb = const.tile([128, 48], F32, tag="bias")
        ident_sb = const.tile([128, 128], BF16, tag="ident")
        fcw_sb = const.tile([128, 12], BF16, tag="fcw")
        fcb_sb = const.tile([3, 1], F32, tag="fcb")
        for l in range(3):
            nc.sync.dma_start(wih_sb[l][:], wih[l].ap())
            nc.sync.dma_start(whh_sb[l][:], whh[l].ap())
        nc.sync.dma_start(bias_sb[:], bias_d.ap())
        nc.sync.dma_start(ident_sb[:], ident_d.ap())
        nc.sync.dma_start(fcw_sb[:], fcw_d.ap())
        nc.sync.dma_start(fcb_sb[:], fcb_d.ap())

        big = ctx.enter_context(tc.tile_pool(name="big", bufs=1))
        g_buf = big.tile([128, 2 * 16 * CW], BF16, tag="gbuf")     # 4MB
        in_buf = big.tile([128, 4 * 4 * CW], BF16, tag="inbuf")    # 2MB
        h_stage = big.tile([128, 2 * 4 * CW], BF16, tag="hstage")  # 1MB
        c_t = big.tile([128, 128], F32, tag="cstate")

        g3 = g_buf[:].rearrange("p (s c) -> p s c", c=CW)    # [128, 32, CW]
        # slot = grp*8 + gi*4 + j: gi=0 -> gates g,f (psum tile0),
        # gi=1 -> gates i,o (tile1)
        g6 = g_buf[:].rearrange("p (par grp gi j c) -> p par grp gi j c",
                                par=2, grp=2, gi=2, j=4)
        i3 = in_buf[:].rearrange("p (b c) -> p b c", c=CW)   # [128, 16, CW]
        h3 = h_stage[:].rearrange("p (x c) -> p x c", c=CW)  # [128, 8, CW]

        ew = ctx.enter_context(tc.tile_pool(name="ew", bufs=2))
        # 3 buffers: the start=True preload of step s+1 WARs against the
        # o-gate ACT reads of step s-2 (instead of s-1), giving a full step
        # of slack. 2 tags x 3 bufs + pa 2 = 8 PSUM banks; fc shares pa.
        ps_rec = ctx.enter_context(tc.tile_pool(name="psr", bufs=3, space="PSUM"))
        ps_pa = ctx.enter_context(tc.tile_pool(name="psa", bufs=2, space="PSUM"))

        def phase_a_slot(l, slot, in_base, g_base, in_ap):
            """G[slot] for one chunk: Kc matmuls (N=CW) + bias ACT."""
            kc = KCS[l]
            ps = ps_pa.tile([128, CW], F32, tag="pa")
            for k in range(kc):
                nc.tensor.matmul(
                    ps[:],
                    lhsT=wih_sb[l][:, k * 2048 + _slot_row(slot):
                                   k * 2048 + _slot_row(slot) + 128],
                    rhs=in_ap(in_base + k),
                    start=(k == 0), stop=(k == kc - 1),
                )
            nc.scalar.activation(
                g3[:, bass.ds(g_base + slot, 1), :].rearrange("p a c -> p (a c)"),
                ps[:], AF.Identity, bias=bias_sb[:, l * 16 + slot: l * 16 + slot + 1])

        def rec_step(l, s, g_base, h_rd, h_wr, pa_emit):
            """One recurrence time step; h_rd/h_wr are h3 block bases.

            Slot order is g(0-3) i(4-7) f(8-11) o(12-15); after each gate's
            4 slots finish, its activation is issued so the c/h chain
            overlaps the remaining matmuls.
            """
            # Two PSUM tiles per step: tile0 = g|f, tile1 = i|o. The pairing
            # interleaves so each gate's ACT evacuation overlaps the other
            # tile's matmul block instead of stalling its own tile's writers.
            ps0 = ps_rec.tile([128, 256], F32, tag="rec0", name="ps0")
            ps1 = ps_rec.tile([128, 256], F32, tag="rec1", name="ps1")
            # blk (g,i,f,o) -> (tile, col base)
            pmap = [(ps0, 0), (ps1, 0), (ps0, 128), (ps1, 128)]
            # PSUM preload G via identity matmuls: no h dependency, so these
            # matmuls keep the PE busy while the previous step's h chain
            # drains. Exactly ONE start=True matmul per tile: start marks the
            # whole 2KB PSUM bank pending-zero, so a second start in the same
            # bank would wipe the first preload's values.
            par = g_base // 16
            for t, pt in enumerate((ps0, ps1)):
                nc.tensor.matmul(
                    pt[:].rearrange("p (g a c) -> p g a c", g=2, c=BL),
                    lhsT=ident_sb[:],
                    rhs=g6[:, bass.ds(par, 1), :, bass.ds(t, 1), :,
                           s * BL:(s + 1) * BL],
                    start=True, stop=False, skip_group_check=True,
                )
            # h[t-1]: last slot of the other-parity buffer for s=0,
            # else slot s-1 of the current chunk's buffer
            hp_base = h_rd if s == 0 else h_wr
            hp_col = ((SC - 1) if s == 0 else (s - 1)) * BL
            gt = ew.tile([128, 512], F32, tag="gates")
            t1 = ew.tile([128, 128], F32, tag="t1")
            t2 = ew.tile([128, 128], F32, tag="t2")
            th = ew.tile([128, 128], F32, tag="th")

            def acc_slot(slot):
                blk, j = slot // 4, slot % 4
                pt, pc = pmap[blk]
                for k in range(4):
                    nc.tensor.matmul(
                        pt[:, pc + j * BL:pc + (j + 1) * BL],
                        lhsT=whh_sb[l][:, k * 2048 + _slot_row(slot):
                                       k * 2048 + _slot_row(slot) + 128],
                        rhs=h3[:, bass.ds(hp_base + k, 1),
                               hp_col:hp_col + BL].rearrange("p a c -> p (a c)"),
                        start=False, stop=(k == 3), skip_group_check=True,
                    )

            for slot in range(4):                                         # g
                acc_slot(slot)
            nc.scalar.activation(gt[:, 0:128], ps0[:, 0:128], AF.Tanh)
            for slot in range(4, 8):                                      # i
                acc_slot(slot)
            nc.scalar.activation(gt[:, 128:256], ps1[:, 0:128], AF.Sigmoid)
            nc.vector.tensor_mul(t1[:], gt[:, 128:256], gt[:, 0:128])     # i*g
            for slot in range(8, 12):                                     # f
                acc_slot(slot)
            nc.scalar.activation(gt[:, 256:384], ps0[:, 128:256], AF.Sigmoid)
            nc.vector.tensor_mul(t2[:], gt[:, 256:384], c_t[:])           # f*c
            nc.vector.tensor_add(c_t[:], t1[:], t2[:])
            nc.scalar.activation(th[:], c_t[:], AF.Tanh)
            # o: sigmoid per slot + h per feature block right after that
            # slot's matmuls; block 0 first, unblocking the next step's
            # first matmul early
            for b in range(4):
                acc_slot(12 + b)
                nc.scalar.activation(gt[:, 384 + b * BL:384 + (b + 1) * BL],
                                     ps1[:, 128 + b * BL:128 + (b + 1) * BL],
                                     AF.Sigmoid)
                nc.vector.tensor_mul(
                    h3[:, bass.ds(h_wr + b, 1), s * BL:(s + 1) * BL]
                    .rearrange("p a c -> p (a c)"),
                    gt[:, 384 + b * BL:384 + (b + 1) * BL],
                    th[:, b * BL:(b + 1) * BL])
            if pa_emit is not None:
                pa_emit(s)

        for l in range(3):
            in_dram = xt.ap() if l == 0 else hdr[:]
            kc = KCS[l]

            # prologue: In chunks 0,1 -> bufs 0,1 ; G chunk 0 -> parity 0
            nc.sync.dma_start(i3[:, 0:kc, :], in_dram[:, :, 0:CW])
            nc.sync.dma_start(i3[:, kc:2 * kc, :], in_dram[:, :, CW:2 * CW])
            for slot in range(16):
                phase_a_slot(l, slot, 0, 0,
                             lambda idx: i3[:, bass.ds(idx, 1), :]
                             .rearrange("p a c -> p (a c)"))
            nc.vector.memset(c_t[:], 0.0)
            nc.vector.memset(h3[:, bass.ds(4, 4), (SC - 1) * BL: SC * BL], 0.0)

            def body(ov, u, l=l, kc=kc, in_dram=in_dram):
                # chunk index civ = ov*UNROLL + u; all mod-2/mod-4 parities
                # depend only on u -> compile-time static APs
                p2 = u & 1
                q2 = (u + 1) & 1
                ld_buf = ((u + 2) & 3) * kc
                use_buf = ((u + 1) & 3) * kc
                nc.sync.dma_start(
                    i3[:, bass.ds(ld_buf, kc), :],
                    in_dram[:, :, bass.ds((ov * UNROLL + u + 2) * CW, CW)])

                def pa_emit(s, l=l, use_buf=use_buf, q2=q2):
                    phase_a_slot(l, s, use_buf, q2 * 16,
                                 lambda idx: i3[:, bass.ds(idx, 1), :]
                                 .rearrange("p a c -> p (a c)"))

                for s in range(SC):
                    rec_step(l, s, p2 * 16, q2 * 4, p2 * 4, pa_emit)
                if l < 2:
                    nc.sync.dma_start(
                        hdr[:, :, bass.ds((ov * UNROLL + u) * CW, CW)],
                        h3[:, bass.ds(p2 * 4, 4), :])

            with tc.For_i(0, NCH // UNROLL, 1) as ov:
                for u in range(UNROLL):
                    body(ov, u)

        # final linear head: out.T [3, BL] = fcW @ h_last (+ fcB)
        hb = ((NCH - 1) & 1) * 4
        ps = ps_pa.tile([3, BL], F32, tag="pa", name="fc_ps")
        for k in range(4):
            nc.tensor.matmul(
                ps[:], lhsT=fcw_sb[:, k * 3:(k + 1) * 3],
                rhs=h3[:, bass.ds(hb + k, 1), (SC - 1) * BL: SC * BL]
                .rearrange("p a c -> p (a c)"),
                start=(k == 0), stop=(k == 3))
        ob = ew.tile([3, BL], F32, tag="out")
        nc.scalar.activation(ob[:], ps[:], AF.Identity, bias=fcb_sb[:])
        nc.sync.dma_start(out_d.ap(), ob[:])

    nc.compile()
    return nc


def _prep(inputs):
    """Host-side layout prep. Returns per-core in_maps."""
    bf = ml_dtypes.bfloat16
    x = np.asarray(inputs["x"], np.float32)
    wihs = [np.asarray(inputs[f"Wih{l}"], np.float32) for l in range(3)]
    whhs = [np.asarray(inputs[f"Whh{l}"], np.float32) for l in range(3)]

    def wt_pack(w, kcs):  # [2048, K] -> [128, kcs*2048]
        return np.ascontiguousarray(
            w.T.reshape(kcs, 128, 2048).transpose(1, 0, 2)
            .reshape(128, kcs * 2048)).astype(bf)

    shared = {}
    for l in range(3):
        shared[f"wih{l}"] = wt_pack(wihs[l], KCS[l])
        shared[f"whh{l}"] = wt_pack(whhs[l], 4)
    shared["ident"] = np.eye(128, dtype=bf)
    bias = np.zeros((128, 48), np.float32)
    for l in range(3):
        bl_ = (np.asarray(inputs[f"bih{l}"], np.float32)
               + np.asarray(inputs[f"bhh{l}"], np.float32))
        for slot in range(16):
            r = _slot_row(slot)
            bias[:, l * 16 + slot] = bl_[r:r + 128]
    shared["bias"] = bias
    shared["fcw"] = np.ascontiguousarray(
        np.asarray(inputs["fcW"], np.float32).T.reshape(4, 128, 3)
        .transpose(1, 0, 2).reshape(128, 12)).astype(bf)
    shared["fcb"] = np.asarray(inputs["fcB"], np.float32).reshape(3, 1)

    in_maps = []
    for c in range(NCORES):
        xc = x[c * BL:(c + 1) * BL]                       # [32, 512, 256]
        xp = xc.transpose(2, 1, 0).reshape(2, 128, TOT)   # [2,128,16384]
        xp = np.ascontiguousarray(xp.transpose(1, 0, 2))  # [128,2,16384]
        xp = np.concatenate(
            [xp, np.zeros((128, 2, SLACK), np.float32)], axis=2).astype(bf)
        in_maps.append({"x_t": xp, **shared})
    return in_maps


_NC_CACHE = None


def kernel(**inputs):
    global _NC_CACHE
    if _NC_CACHE is None:
        _NC_CACHE = _build()
    nc = _NC_CACHE
    in_maps = _prep(inputs)
    res = run_bass_kernel_spmd(nc, in_maps, core_ids=list(range(NCORES)))
    out = np.empty((B, O), np.float32)
    for c in range(NCORES):
        out[c * BL:(c + 1) * BL] = res.results[c]["out"].T
    return out


# revision 30
# speedup vs baseline: 1.1651x; 1.0249x over previous
"""3-layer LSTM (B=256,T=512,I=256,H=512) + linear head on 8 NeuronCores.

Strategy: data-parallel over batch (32/core). Per layer, the input-side
matmul G = Wih @ x_t (+ biases) for a *chunk* of future time steps is
computed at full PE efficiency (N=512 streams) and interleaved with the
sequential h-recurrence of the current chunk; G never leaves SBUF.
Gate layout: gates.T packed as 4 PSUM tiles (one per gate g|i|f|o) of
[128 part, 128 cols = 4 feature slots x 32 batch] per step. Separate
tiles keep the per-gate ACT evacuations from serializing against the
next gate's matmuls. G is preloaded into PSUM by identity matmuls
(I.T @ G, start=True) on the PE itself -- no DVE copy on the critical
path, and the 16 preload matmuls pad the PE pipe while the previous
step's h chain drains. Each step then runs 64 weight-stationary bf16
accumulate matmuls (K=128, M=128, N=32). ACT evacuation is split per
gate block (tanh g, sigmoid i, f) and per slot for o, with tanh(c)
issued between f and o, so the c/h elementwise chain overlaps the tail
of the matmul block and h feature block 0 lands as early as possible.
The chunk loop is unrolled 4x inside the hardware loop so all SBUF
access patterns are static (no per-matmul register-AP setup on the PE
queue). c stays fp32-resident; h is written bf16 per feature block
into the layout the next matmul and the next layer's batched input
matmul consume.
"""

import numpy as np
import ml_dtypes
from contextlib import ExitStack

import concourse.bass as bass
import concourse.bacc as bacc
import concourse.tile as tile
from concourse import mybir
from concourse.bass_utils import run_bass_kernel_spmd

BF16 = mybir.dt.bfloat16
FP8 = mybir.dt.float8e4
F32 = mybir.dt.float32
AF = mybir.ActivationFunctionType
WSCALE = 32.0   # weights pre-scaled x32 (fp8 subnormal avoidance); gate
                # ACTs descale by 1/32 (exact, power of 2)

B, T, I, H, O = 256, 512, 256, 512, 3
NCORES = 8
BL = B // NCORES          # 32 batch rows per core
SC = 16                   # time steps per chunk
CW = SC * BL              # 512 cols per chunk
NCH = T // SC             # 32 chunks
UNROLL = 4                # chunks per hardware-loop iteration
TOT = T * BL              # 16384 cols total
SLACK = 2 * CW            # prefetch overrun slack (cols)

# gate blocks in psum-slot order: g | i | f | o  (slot = blk*4 + j)
# block -> base row in the canonical (i,f,g,o) 2048 gate layout
GATE_BASE = [1024, 0, 512, 1536]   # g, i, f, o
KCS = [2, 4, 4]                    # K chunks per layer (256, 512, 512)


def _slot_row(slot):
    return GATE_BASE[slot // 4] + 128 * (slot % 4)


def _build():
    nc = bacc.Bacc("TRN2", target_bir_lowering=False, debug=False,
                   num_devices=NCORES)

    xt = nc.dram_tensor("x_t", (128, 2, TOT + SLACK), BF16, kind="ExternalInput")
    wih = [nc.dram_tensor(f"wih{l}", (128, KCS[l] * 2048), BF16,
                          kind="ExternalInput") for l in range(3)]
    whh = [nc.dram_tensor(f"whh{l}", (128, 4 * 2048), BF16,
                          kind="ExternalInput") for l in range(3)]
    bias_d = nc.dram_tensor("bias", (128, 48), F32, kind="ExternalInput")
    ident_d = nc.dram_tensor("ident", (128, 128), BF16, kind="ExternalInput")
    fcw_d = nc.dram_tensor("fcw", (128, 12), BF16, kind="ExternalInput")
    fcb_d = nc.dram_tensor("fcb", (3, 1), F32, kind="ExternalInput")
    out_d = nc.dram_tensor("out", (3, BL), F32, kind="ExternalOutput")

    with tile.TileContext(nc) as tc, ExitStack() as ctx:
        dram = ctx.enter_context(tc.tile_pool(name="dram", bufs=1, space="DRAM"))
        hdr = dram.tile([128, 4, TOT + SLACK], BF16)   # inter-layer H seq

        const = ctx.enter_context(tc.tile_pool(name="const", bufs=1))
        wih_sb = [const.tile([128, KCS[l] * 2048], BF16, tag=f"wih{l}",
                             name=f"wih_sb{l}") for l in range(3)]
        whh_sb = [const.tile([128, 4 * 2048], BF16, tag=f"whh{l}",
                             name=f"whh_sb{l}") for l in range(3)]
        bias_sb = const.tile([128, 48], F32, tag="bias")
        ident_sb = const.tile([128, 128], BF16, tag="ident")
        fcw_sb = const.tile([128, 12], BF16, tag="fcw")
        fcb_sb = const.tile([3, 1], F32, tag="fcb")
        for l in range(3):
            nc.sync.dma_start(wih_sb[l][:], wih[l].ap())
            nc.sync.dma_start(whh_sb[l][:], whh[l].ap())
        nc.sync.dma_start(bias_sb[:], bias_d.ap())
        nc.sync.dma_start(ident_sb[:], ident_d.ap())
        nc.sync.dma_start(fcw_sb[:], fcw_d.ap())
        nc.sync.dma_start(fcb_sb[:], fcb_d.ap())

        big = ctx.enter_context(tc.tile_pool(name="big", bufs=1))
        g_buf = big.tile([128, 2 * 16 * CW], BF16, tag="gbuf")     # 4MB
        in_buf = big.tile([128, 4 * 4 * CW], BF16, tag="inbuf")    # 2MB
        h_stage = big.tile([128, 2 * 4 * CW], BF16, tag="hstage")  # 1MB
        c_t = big.tile([128, 128], F32, tag="cstate")

        g3 = g_buf[:].rearrange("p (s c) -> p s c", c=CW)    # [128, 32, CW]
        # slot = grp*8 + gi*4 + j: gi=0 -> gates g,f (psum tile0),
        # gi=1 -> gates i,o (tile1)
        g6 = g_buf[:].rearrange("p (par grp gi j c) -> p par grp gi j c",
                                par=2, grp=2, gi=2, j=4)
        i3 = in_buf[:].rearrange("p (b c) -> p b c", c=CW)   # [128, 16, CW]
        h3 = h_stage[:].rearrange("p (x c) -> p x c", c=CW)  # [128, 8, CW]

        ew = ctx.enter_context(tc.tile_pool(name="ew", bufs=2))
        # 3 buffers: the start=True preload of step s+1 WARs against the
        # o-gate ACT reads of step s-2 (instead of s-1), giving a full step
        # of slack. 2 tags x 3 bufs + pa 2 = 8 PSUM banks; fc shares pa.
        ps_rec = ctx.enter_context(tc.tile_pool(name="psr", bufs=3, space="PSUM"))
        ps_pa = ctx.enter_context(tc.tile_pool(name="psa", bufs=2, space="PSUM"))

        def phase_a_slot(l, slot, in_base, g_base):
            """G[slot] for one chunk: Kc matmuls (N=CW) + bias ACT."""
            kc = KCS[l]
            row = _slot_row(slot)
            ps = ps_pa.tile([128, CW], F32, tag="pa")
            for k in range(kc):
                nc.tensor.matmul(
                    ps[:],
                    lhsT=wih_sb[l][:, k * 2048 + row: k * 2048 + row + 128],
                    rhs=i3[:, bass.ds(in_base + k, 1), :]
                    .rearrange("p a c -> p (a c)"),
                    start=(k == 0), stop=(k == kc - 1),
                )
            nc.scalar.activation(
                g3[:, bass.ds(g_base + slot, 1), :].rearrange("p a c -> p (a c)"),
                ps[:], AF.Identity, bias=bias_sb[:, l * 16 + slot: l * 16 + slot + 1])

        def rec_step(l, s, g_base, h_rd, h_wr, pa_emit):
            """One recurrence time step; h_rd/h_wr are h3 block bases.

            Slot order is g(0-3) i(4-7) f(8-11) o(12-15); after each gate's
            4 slots finish, its activation is issued so the c/h chain
            overlaps the remaining matmuls.
            """
            # Two PSUM tiles per step: tile0 = g|f, tile1 = i|o. The pairing
            # interleaves so each gate's ACT evacuation overlaps the other
            # tile's matmul block instead of stalling its own tile's writers.
            ps0 = ps_rec.tile([128, 256], F32, tag="rec0", name="ps0")
            ps1 = ps_rec.tile([128, 256], F32, tag="rec1", name="ps1")
            # blk (g,i,f,o) -> (tile, col base)
            pmap = [(ps0, 0), (ps1, 0), (ps0, 128), (ps1, 128)]
            # PSUM preload G via DVE copies: off the PE queue entirely; the
            # phase-A matmuls pad the PE while these run, and bufs=3 gives a
            # full step of WAR slack vs the previous reads of the bank.
            par = g_base // 16
            for t, pt in enumerate((ps0, ps1)):
                nc.vector.tensor_copy(
                    pt[:].rearrange("p (g a c) -> p g a c", g=2, c=BL),
                    g6[:, bass.ds(par, 1), :, bass.ds(t, 1), :,
                       s * BL:(s + 1) * BL])
            # h[t-1]: last slot of the other-parity buffer for s=0,
            # else slot s-1 of the current chunk's buffer
            hp_base = h_rd if s == 0 else h_wr
            hp_col = ((SC - 1) if s == 0 else (s - 1)) * BL
            gt = ew.tile([128, 512], F32, tag="gates")
            t1 = ew.tile([128, 128], F32, tag="t1")
            t2 = ew.tile([128, 128], F32, tag="t2")
            th = ew.tile([128, 128], F32, tag="th")

            def acc_slot(slot):
                blk, j = slot // 4, slot % 4
                pt, pc = pmap[blk]
                for k in range(4):
                    nc.tensor.matmul(
                        pt[:, pc + j * BL:pc + (j + 1) * BL],
                        lhsT=whh_sb[l][:, k * 2048 + _slot_row(slot):
                                       k * 2048 + _slot_row(slot) + 128],
                        rhs=h3[:, bass.ds(hp_base + k, 1),
                               hp_col:hp_col + BL].rearrange("p a c -> p (a c)"),
                        start=False, stop=(k == 3), skip_group_check=True,
                    )

            for slot in range(4):                                         # g
                acc_slot(slot)
            nc.scalar.activation(gt[:, 0:128], ps0[:, 0:128], AF.Tanh)
            for slot in range(4, 8):                                      # i
                acc_slot(slot)
            nc.scalar.activation(gt[:, 128:256], ps1[:, 0:128], AF.Sigmoid)
            nc.vector.tensor_mul(t1[:], gt[:, 128:256], gt[:, 0:128])     # i*g
            for slot in range(8, 12):                                     # f
                acc_slot(slot)
            nc.scalar.activation(gt[:, 256:384], ps0[:, 128:256], AF.Sigmoid)
            nc.vector.tensor_mul(t2[:], gt[:, 256:384], c_t[:])           # f*c
            nc.vector.tensor_add(c_t[:], t1[:], t2[:])
            nc.scalar.activation(th[:], c_t[:], AF.Tanh)
            # o: sigmoid per slot + h per feature block right after that
            # slot's matmuls; block 0 first, unblocking the next step's
            # first matmul early
            for b in range(4):
                acc_slot(12 + b)
                nc.scalar.activation(gt[:, 384 + b * BL:384 + (b + 1) * BL],
                                     ps1[:, 128 + b * BL:128 + (b + 1) * BL],
                                     AF.Sigmoid)
                nc.vector.tensor_mul(
                    h3[:, bass.ds(h_wr + b, 1), s * BL:(s + 1) * BL]
                    .rearrange("p a c -> p (a c)"),
                    gt[:, 384 + b * BL:384 + (b + 1) * BL],
                    th[:, b * BL:(b + 1) * BL])
            if pa_emit is not None:
                pa_emit(s)

        for l in range(3):
            in_dram = xt.ap() if l == 0 else hdr[:]
            kc = KCS[l]

            # prologue: In chunks 0,1 -> bufs 0,1 ; G chunk 0 -> parity 0
            nc.sync.dma_start(i3[:, 0:kc, :], in_dram[:, :, 0:CW])
            nc.sync.dma_start(i3[:, kc:2 * kc, :], in_dram[:, :, CW:2 * CW])
            for slot in range(16):
                phase_a_slot(l, slot, 0, 0)
            nc.vector.memset(c_t[:], 0.0)
            nc.vector.memset(h3[:, bass.ds(4, 4), (SC - 1) * BL: SC * BL], 0.0)

            def body(ov, u, l=l, kc=kc, in_dram=in_dram):
                # chunk index civ = ov*UNROLL + u; all mod-2/mod-4 parities
                # depend only on u -> compile-time static APs
                p2 = u & 1
                q2 = (u + 1) & 1
                ld_buf = ((u + 2) & 3) * kc
                use_buf = ((u + 1) & 3) * kc
                nc.sync.dma_start(
                    i3[:, bass.ds(ld_buf, kc), :],
                    in_dram[:, :, bass.ds((ov * UNROLL + u + 2) * CW, CW)])

                def pa_emit(s, l=l, use_buf=use_buf, q2=q2):
                    phase_a_slot(l, s, use_buf, q2 * 16)

                for s in range(SC):
                    rec_step(l, s, p2 * 16, q2 * 4, p2 * 4, pa_emit)
                if l < 2:
                    nc.sync.dma_start(
                        hdr[:, :, bass.ds((ov * UNROLL + u) * CW, CW)],
                        h3[:, bass.ds(p2 * 4, 4), :])

            with tc.For_i(0, NCH // UNROLL, 1) as ov:
                for u in range(UNROLL):
                    body(ov, u)

        # final linear head: out.T [3, BL] = fcW @ h_last (+ fcB)
        hb = ((NCH - 1) & 1) * 4
        ps = ps_pa.tile([3, BL], F32, tag="pa", name="fc_ps")
        for k in range(4):
            nc.tensor.matmul(
                ps[:], lhsT=fcw_sb[:, k * 3:(k + 1) * 3],
                rhs=h3[:, bass.ds(hb + k, 1), (SC - 1) * BL: SC * BL]
                .rearrange("p a c -> p (a c)"),
                start=(k == 0), stop=(k == 3))
        ob = ew.tile([3, BL], F32, tag="out")
        nc.scalar.activation(ob[:], ps[:], AF.Identity, bias=fcb_sb[:])
        nc.sync.dma_start(out_d.ap(), ob[:])

    nc.compile()
    return nc


def _prep(inputs):
    """Host-side layout prep. Returns per-core in_maps."""
    bf = ml_dtypes.bfloat16
    x = np.asarray(inputs["x"], np.float32)
    wihs = [np.asarray(inputs[f"Wih{l}"], np.float32) for l in range(3)]
    whhs = [np.asarray(inputs[f"Whh{l}"], np.float32) for l in range(3)]

    def wt_pack(w, kcs):  # [2048, K] -> [128, kcs*2048]
        return np.ascontiguousarray(
            w.T.reshape(kcs, 128, 2048).transpose(1, 0, 2)
            .reshape(128, kcs * 2048)).astype(bf)

    shared = {}
    for l in range(3):
        shared[f"wih{l}"] = wt_pack(wihs[l], KCS[l])
        shared[f"whh{l}"] = wt_pack(whhs[l], 4)
    shared["ident"] = np.eye(128, dtype=bf)
    bias = np.zeros((128, 48), np.float32)
    for l in range(3):
        bl_ = (np.asarray(inputs[f"bih{l}"], np.float32)
               + np.asarray(inputs[f"bhh{l}"], np.float32))
        for slot in range(16):
            r = _slot_row(slot)
            bias[:, l * 16 + slot] = bl_[r:r + 128]
    shared["bias"] = bias
    shared["fcw"] = np.ascontiguousarray(
        np.asarray(inputs["fcW"], np.float32).T.reshape(4, 128, 3)
        .transpose(1, 0, 2).reshape(128, 12)).astype(bf)
    shared["fcb"] = np.asarray(inputs["fcB"], np.float32).reshape(3, 1)

    in_maps = []
    for c in range(NCORES):
        xc = x[c * BL:(c + 1) * BL]                       # [32, 512, 256]
        xp = xc.transpose(2, 1, 0).reshape(2, 128, TOT)   # [2,128,16384]
        xp = np.ascontiguousarray(xp.transpose(1, 0, 2))  # [128,2,16384]
        xp = np.concatenate(
            [xp, np.zeros((128, 2, SLACK), np.float32)], axis=2).astype(bf)
        in_maps.append({"x_t": xp, **shared})
    return in_maps


_NC_CACHE = None


def kernel(**inputs):
    global _NC_CACHE
    if _NC_CACHE is None:
        _NC_CACHE = _build()
    nc = _NC_CACHE
    in_maps = _prep(inputs)
    res = run_bass_kernel_spmd(nc, in_maps, core_ids=list(range(NCORES)))
    out = np.empty((B, O), np.float32)
    for c in range(NCORES):
        out[c * BL:(c + 1) * BL] = res.results[c]["out"].T
    return out
